# revision 31
# baseline (speedup 1.0000x reference)
"""Trainium2 Bass kernel for the DGTreg soft-decision-tree module.

Math shortcut exploited (vs naive reference):
  - The fixed +-1 "and" matrix encodes a perfect binary tree of height 8.
    For each sample the post-sparser routing weight is a one-hot over the
    256 leaves at the sign-descent leaf, with value v = max softmax prob.
  - and_z[b,l] = fac_b * c[b,l] with c = sign(pred_z) @ Wand.T in {-8..8};
    the unique maximum c==8 identifies the leaf, and the softmax max has
    the closed form v = sigmoid(2*fac)^8 (all equal-depth subtree sums of
    exp(+-fac) coincide, so the denominator is (e^fac + e^-fac)^8).
  - out[b,o] = v * <x[b], Wor[o,:,l*]> + <x[b], bor[o,:]>
    std[b,o] = clip(v * action_stds[l*,o], -20, 2)
  The one-hot selection of Wor[:,:,l*] is computed as a matmul of the 0/1
  indicator against a [leaf, (o,i)] re-layout of Wor (float32r, 1 cyc/row),
  and the final i-contraction is a DVE multiply + reduce.

Sharding: pure data parallel, batch 65536 split across 8 cores.
"""

import sys

try:
    import concourse.bass as bass  # noqa: F401
except ImportError:
    sys.path.insert(0, "/opt/trn_rl_repo")

import numpy as np
import ml_dtypes

import concourse.bass as bass
import concourse.bacc as bacc
import concourse.tile as tile
import concourse.mybir as mybir
from concourse import bass_utils
from concourse.masks import make_identity

F32 = mybir.dt.float32
F32R = mybir.dt.float32r
BF16 = mybir.dt.bfloat16
AF = mybir.ActivationFunctionType
ALU = mybir.AluOpType

N_CORES = 8
B_FULL = 65536
BC = B_FULL // N_CORES       # 8192 rows per core
BT = 512                     # samples per outer tile
NT = BC // BT                # 16 outer tiles
NS = BT // 128               # 4 sub-tiles of 128 samples
IN_DIM = 128
NODES = 255
LEAF = 256
OUT = 8

_CACHE = {}

# pool-size knobs (swept via TimelineSim)
BUFS_WORK = 3
BUFS_TFP = 3
BUFS_FE = 2
BUFS_WS = 4
BUFS_SM = 2
PIPE_DEPTH = 2


def _bcast_free(ap, n, at=1):
    """Insert a stride-0 (broadcast) free dim of size n at position `at`."""
    new = list(list(p) for p in ap.ap)
    new.insert(at, [0, n])
    return bass.AP(tensor=ap.tensor, offset=ap.offset, ap=new)


def _build():
    nc = bacc.Bacc("TRN2", target_bir_lowering=False, debug=False,
                   num_devices=N_CORES)

    x_d = nc.dram_tensor("x", [BC, IN_DIM], F32, kind="ExternalInput")
    wpt_d = nc.dram_tensor("wpt", [IN_DIM, NODES], F32, kind="ExternalInput")
    bp_d = nc.dram_tensor("bp", [256, 1], F32, kind="ExternalInput")
    wandt_d = nc.dram_tensor("wandt", [256, LEAF], BF16, kind="ExternalInput")
    wor2_d = nc.dram_tensor("wor2", [LEAF, OUT * IN_DIM], F32R,
                            kind="ExternalInput")
    onesr_d = nc.dram_tensor("onesr", [IN_DIM, 2], F32R, kind="ExternalInput")
    astd_d = nc.dram_tensor("astd", [LEAF, OUT], F32R, kind="ExternalInput")
    bort_d = nc.dram_tensor("bort", [IN_DIM, OUT], F32, kind="ExternalInput")
    out_d = nc.dram_tensor("out", [BC, OUT], F32, kind="ExternalOutput")
    std_d = nc.dram_tensor("std", [BC, OUT], F32, kind="ExternalOutput")

    with tile.TileContext(nc) as tc:
        with (
            tc.tile_pool(name="consts", bufs=1) as consts,
            tc.tile_pool(name="work", bufs=BUFS_WORK) as work,
            tc.tile_pool(name="tfp", bufs=BUFS_TFP) as tfp,
            tc.tile_pool(name="psfe", bufs=BUFS_FE, space="PSUM") as psfe,
            tc.tile_pool(name="psws", bufs=BUFS_WS, space="PSUM") as psws,
            tc.tile_pool(name="pssmall", bufs=BUFS_SM, space="PSUM") as pssmall,
        ):
            # ---- constants ----
            wpt_sb = consts.tile([128, NODES], F32)
            nc.sync.dma_start(wpt_sb[:], wpt_d.ap()[:, :])
            bp_sb = consts.tile([128, 2, 1], F32)
            nc.sync.dma_start(
                bp_sb[:], bp_d.ap().rearrange("(k p) u -> p k u", p=128))
            wandt_sb = consts.tile([128, 2, LEAF], BF16)
            nc.sync.dma_start(
                wandt_sb[:], wandt_d.ap().rearrange("(k p) l -> p k l", p=128))
            wor2_sb = consts.tile([128, 2, OUT * IN_DIM], F32R)
            nc.sync.dma_start(
                wor2_sb[:], wor2_d.ap().rearrange("(k p) c -> p k c", p=128))
            astd_sb = consts.tile([128, 2, OUT], F32R)
            nc.sync.dma_start(
                astd_sb[:], astd_d.ap().rearrange("(k p) o -> p k o", p=128))
            bort_sb = consts.tile([128, OUT], F32)
            nc.sync.dma_start(bort_sb[:], bort_d.ap()[:, :])
            ones8 = consts.tile([128, OUT], F32)
            nc.vector.memset(ones8[:], 2.0 / NODES)
            neg15 = consts.tile([128, 1], F32)
            nc.vector.memset(neg15[:], -15.0)
            ident = consts.tile([128, 128], F32)
            make_identity(nc, ident[:])
            onesr_sb = consts.tile([IN_DIM, 2], F32R)
            nc.sync.dma_start(onesr_sb[:], onesr_d.ap()[:, :])

            def frontend(t):
                """DMA x, transpose, predicate, signs, and-layer, indicator."""
                b0 = t * BT
                x_tile = work.tile([128, NS, IN_DIM], F32, tag="xt",
                                   name=f"x{t}")
                nc.sync.dma_start(
                    x_tile[:],
                    x_d.ap()[b0:b0 + BT, :].rearrange("(s p) i -> p s i",
                                                      p=128))
                # transpose x -> XT [i, (s,b)]
                xt_ps = psfe.tile([128, BT], F32, tag="fe", name=f"xps{t}")
                for s in range(NS):
                    nc.tensor.transpose(xt_ps[:, s * 128:(s + 1) * 128],
                                        x_tile[:, s, :], ident[:])
                XT = work.tile([128, BT], F32, tag="XT", name=f"XT{t}")
                nc.scalar.copy(XT[:], xt_ps[:])

                # predicate layer: P^T [nodes, (s,b)] fp32
                p0_ps = psfe.tile([128, BT], F32, tag="fe", name=f"p0_{t}")
                p1_ps = psfe.tile([128, BT], F32, tag="fe", name=f"p1_{t}")
                nc.tensor.matmul(p0_ps[:], wpt_sb[:, 0:128], XT[:],
                                 start=True, stop=True)
                nc.tensor.matmul(p1_ps[0:127, :], wpt_sb[:, 128:NODES],
                                 XT[:], start=True, stop=True)

                return dict(t=t, b0=b0, x_tile=x_tile, XT=XT,
                            p0_ps=p0_ps, p1_ps=p1_ps)

            def frontend2(st):
                t = st["t"]
                p0_ps, p1_ps = st["p0_ps"], st["p1_ps"]
                S0 = work.tile([128, BT], BF16, tag="S0", name=f"S0_{t}")
                S1 = work.tile([127, BT], BF16, tag="S1", name=f"S1_{t}")
                A0 = work.tile([128, BT], F32, tag="A0", name=f"A0_{t}")
                A1 = work.tile([127, BT], F32, tag="A1", name=f"A1_{t}")
                nc.scalar.activation(S0[:], p0_ps[:], AF.Sign,
                                     bias=bp_sb[:, 0, :])
                nc.scalar.activation(S1[:], p1_ps[0:127, :], AF.Sign,
                                     bias=bp_sb[0:127, 1, :])
                nc.scalar.activation(A0[:], p0_ps[:], AF.Abs,
                                     bias=bp_sb[:, 0, :])
                nc.scalar.activation(A1[:], p1_ps[0:127, :], AF.Abs,
                                     bias=bp_sb[0:127, 1, :])

                # and layer: c^T [leaf, (s,b)] = WandT.T @ S^T  (bf16 exact)
                c_ps = [psfe.tile([128, BT], F32, tag="fe", name=f"c{h}_{t}")
                        for h in range(2)]
                for h in range(2):
                    nc.tensor.matmul(c_ps[h][:],
                                     wandt_sb[0:128, 0, h * 128:(h + 1) * 128],
                                     S0[:], start=True, stop=False)
                    nc.tensor.matmul(c_ps[h][:],
                                     wandt_sb[0:127, 1, h * 128:(h + 1) * 128],
                                     S1[:], start=False, stop=True)
                # 0/1 leaf indicator: relu(2c - 15)
                I0 = work.tile([128, BT], F32R, tag="I0", name=f"I0_{t}")
                I1 = work.tile([128, BT], F32R, tag="I1", name=f"I1_{t}")
                nc.scalar.activation(I0[:], c_ps[0][:], AF.Relu,
                                     bias=neg15[:], scale=2.0)
                nc.scalar.activation(I1[:], c_ps[1][:], AF.Relu,
                                     bias=neg15[:], scale=2.0)
                st.update(A0=A0, A1=A1, I0=I0, I1=I1)
                return st

            def backend(st):
                t, b0 = st["t"], st["b0"]
                XT, A0, A1 = st["XT"], st["A0"], st["A1"]
                I0, I1 = st["I0"], st["I1"]
                # psum: fac (cols 0:8), xbias (8:16), std raw (16:24),
                # or-dots (24:40, stride 2)
                sm_ps = pssmall.tile([128, NS, 40], F32, tag="sm",
                                     name=f"sm{t}")
                def emit_fac_v():
                    for s in range(NS):
                        sl = slice(s * 128, (s + 1) * 128)
                        nc.tensor.matmul(sm_ps[:, s, 0:8], A0[:, sl],
                                         ones8[:], start=True, stop=False)
                        nc.tensor.matmul(sm_ps[:, s, 0:8], A1[:, sl],
                                         ones8[0:127, :], start=False,
                                         stop=True)
                    v1 = work.tile([128, NS, OUT], F32, tag="v1",
                                   name=f"v1_{t}")
                    v2 = work.tile([128, NS, OUT], F32, tag="v2",
                                   name=f"v2_{t}")
                    v4 = work.tile([128, NS, OUT], F32, tag="v4",
                                   name=f"v4_{t}")
                    v8 = work.tile([128, NS, OUT], F32, tag="v8",
                                   name=f"v8_{t}")
                    nc.scalar.activation(v1[:], sm_ps[:, :, 0:8], AF.Sigmoid)
                    nc.scalar.activation(v2[:], v1[:], AF.Square)
                    nc.scalar.activation(v4[:], v2[:], AF.Square)
                    nc.scalar.activation(v8[:], v4[:], AF.Square)
                    return v8

                # or-layer, feature-major per output o (sw-pipelined):
                #   wsel_f[i, b] = sum_l Wor2[l, (o,i)] * I[l, b]   (PE)
                #   tmp_f = wsel_f * XT                             (DVE, f32r)
                #   dot[b, o] = sum_i tmp_f[i, b]                   (PE, N=2)
                def or_mm(o):
                    wsel_f = psws.tile([128, 512], F32, tag="ws",
                                       name=f"ws{t}_{o}")
                    for kt in range(2):
                        nc.tensor.matmul(
                            wsel_f[:],
                            wor2_sb[:, kt, o * 128:(o + 1) * 128],
                            (I0 if kt == 0 else I1)[:],
                            start=(kt == 0), stop=(kt == 1))
                    return wsel_f

                def or_mult(o, wsel_f):
                    tmp_f = tfp.tile([128, BT], F32R, tag="tmpf",
                                     name=f"tf{t}_{o}")
                    nc.vector.tensor_tensor(tmp_f[:], wsel_f[:], XT[:],
                                            ALU.mult)
                    return tmp_f

                def or_red(o, tmp_f):
                    for s in range(NS):
                        nc.tensor.matmul(
                            sm_ps[:, s, 24 + 2 * o:26 + 2 * o],
                            tmp_f[:, s * 128:(s + 1) * 128], onesr_sb[:],
                            start=True, stop=True)

                wprev = or_mm(0)
                tprev = None
                for o in range(1, OUT):
                    wcur = or_mm(o)
                    tcur = or_mult(o - 1, wprev)
                    if tprev is not None:
                        or_red(o - 2, tprev)
                    wprev, tprev = wcur, tcur
                tcur = or_mult(OUT - 1, wprev)
                or_red(OUT - 2, tprev)
                or_red(OUT - 1, tcur)
                # small matmuls (PE gap fillers)
                for s in range(NS):
                    sl = slice(s * 128, (s + 1) * 128)
                    nc.tensor.matmul(sm_ps[:, s, 8:16], XT[:, sl], bort_sb[:],
                                     start=True, stop=True)
                    for kt in range(2):
                        lhs = (I0 if kt == 0 else I1)[:, sl]
                        nc.tensor.matmul(sm_ps[:, s, 16:24],
                                         lhs, astd_sb[:, kt, :],
                                         start=(kt == 0), stop=(kt == 1))
                v8 = emit_fac_v()
                return dict(t=t, b0=b0, sm_ps=sm_ps, v8=v8)

            def finals(fs):
                t, b0, sm_ps, v8 = fs["t"], fs["b0"], fs["sm_ps"], fs["v8"]
                # out = v * dot + xbias ; std = clip(v * stdraw)
                out_sb = work.tile([128, NS, OUT], F32, tag="out",
                                   name=f"o{t}")
                std_sb = work.tile([128, NS, OUT], F32, tag="std",
                                   name=f"sd{t}")
                dotv = sm_ps[:, :, 24:40].rearrange(
                    "p s (o two) -> p s o two", two=2)[:, :, :, 0]
                nc.vector.tensor_tensor(out_sb[:], dotv, v8[:], ALU.mult)
                nc.vector.tensor_tensor(out_sb[:], out_sb[:],
                                        sm_ps[:, :, 8:16], ALU.add)
                nc.vector.tensor_tensor(std_sb[:], sm_ps[:, :, 16:24], v8[:],
                                        ALU.mult)
                nc.vector.tensor_scalar(std_sb[:], std_sb[:], 2.0, -20.0,
                                        ALU.min, ALU.max)
                nc.sync.dma_start(
                    out_d.ap()[b0:b0 + BT, :].rearrange("(s p) o -> p s o",
                                                        p=128), out_sb[:])
                nc.sync.dma_start(
                    std_d.ap()[b0:b0 + BT, :].rearrange("(s p) o -> p s o",
                                                        p=128), std_sb[:])

            # 2-deep software pipeline: emit frontend(t+1) before backend(t)
            prev = frontend2(frontend(0))
            for t in range(1, NT):
                cur = frontend2(frontend(t))
                finals(backend(prev))
                prev = cur
            finals(backend(prev))

    nc.compile()
    return nc


def _get_nc():
    if "nc" not in _CACHE:
        _CACHE["nc"] = _build()
    return _CACHE["nc"]


def kernel(x, Wp, bp, Wand, Wor, bor, action_stds):
    nc = _get_nc()
    x = np.ascontiguousarray(np.asarray(x, dtype=np.float32))
    Wp = np.asarray(Wp, dtype=np.float32)
    bp = np.asarray(bp, dtype=np.float32)
    Wand = np.asarray(Wand, dtype=np.float32)
    Wor = np.asarray(Wor, dtype=np.float32)
    bor = np.asarray(bor, dtype=np.float32)
    action_stds = np.asarray(action_stds, dtype=np.float32)

    wpt = np.ascontiguousarray(Wp.T)                      # [128, 255]
    bp_pad = np.zeros((256, 1), np.float32)
    bp_pad[:NODES, 0] = bp
    wandt = np.zeros((256, LEAF), ml_dtypes.bfloat16)
    wandt[:NODES] = Wand.T.astype(ml_dtypes.bfloat16)     # exact +-1/0
    wor2 = np.ascontiguousarray(
        Wor.transpose(2, 0, 1).reshape(LEAF, OUT * IN_DIM))
    astd = np.ascontiguousarray(action_stds)              # [256, 8]
    bort = np.ascontiguousarray(bor.T)                    # [128, 8]
    ONESR = np.ones((IN_DIM, 2), np.float32)

    in_maps = []
    for i in range(N_CORES):
        in_maps.append({
            "x": np.ascontiguousarray(x[i * BC:(i + 1) * BC]),
            "wpt": wpt, "bp": bp_pad, "wandt": wandt,
            "wor2": wor2, "astd": astd, "bort": bort,
            "onesr": ONESR,
        })
    res = bass_utils.run_bass_kernel_spmd(nc, in_maps,
                                          core_ids=list(range(N_CORES)))
    out = np.concatenate([r["out"] for r in res.results], axis=0)
    std = np.concatenate([r["std"] for r in res.results], axis=0)
    return out, std


if __name__ == "__main__":
    rng = np.random.default_rng(0)
    d = np.load("/root/problem/work/ref.npz")
    out, std = kernel(d["x"], d["Wp"], d["bp"], d["Wand"], d["Wor"],
                      d["bor"], d["action_stds"])
    for name, got, ref in (("out", out, d["out"]), ("std", std, d["std"])):
        mx = np.abs(got - ref).max() / np.abs(ref).max()
        nm = np.linalg.norm(got - ref) / np.linalg.norm(ref)
        print(f"{name}: max_rel={mx:.3e} norm_rel={nm:.3e}")
